# revision 5
# baseline (speedup 1.0000x reference)
"""EquivSetConv (hypergraph GNN message passing) on 8 Trainium2 NeuronCores.

Math (reference):
    H   = X @ W1_w.T + b1
    Xe  = segment_sum(H[vertex], edges, E)            # [E, 64]
    Xev = concat([X[vertex], Xe[edges]], -1) @ W2_w.T + b2
    Xv  = segment_sum(Xev, vertex, N)                 # [N, 64]
    out = ((1-a)*Xv + a*X0) @ W_w.T + W_b             # a = 0.5
    returns (out, Xe)

Reformulation used here (all matmuls pushed through the segment sums):
    Ye  = segment_sum(X[vertex], edges)               # gather-X + on-chip reduce
    Xe  = Ye @ W1_w.T + cnt (x) b1
    Z   = Xe @ G1.T          with G1 = (1-a) * W_w @ W2b      (W2b = W2_w[:, 64:])
    A'  = segment_sum(Z[edges-of-incidence], vertex)  # gather-Z + on-chip reduce
    out = A' + deg . (X @ G2.T) + c2 (x) deg + a * X0 @ W_w.T + 1 (x) W_b
          with G2 = (1-a) * W_w @ W2a  (W2a = W2_w[:, :64]),  c2 = (1-a) * W_w @ b2

Sharding: edges bin-packed into per-core windows (phase A), nodes bin-packed
into per-core windows (phase C). The two gathers use 4-queue dma_gather;
the segment sums are one-hot (S) matmuls on the PE into transposed PSUM
windows, with S built on-device from per-slot segment ids via broadcast
is_equal. Z is AllGathered between the phases (5.4 MB).
"""

import heapq
import os

import numpy as np

# ---------------------------------------------------------------- constants
N_NODES = 100000
N_EDGES = 20000
D = 64
NC = 8
ALPHA = 0.5

# phase A (edge side)
EB_EDGES = 10        # max edges per bin
EB_SLOTS = 512       # slots per bin (4 blocks of 128)
EB_BLOCKS = 4
WINA_BINS = 12       # bins per window -> 120 edge columns
WINA_EDGES = WINA_BINS * EB_EDGES          # 120
WINA_SLOTS = WINA_BINS * EB_SLOTS          # 6144
WINA_BLOCKS = WINA_BINS * EB_BLOCKS        # 48
WINS_A = 22                                 # windows per core
BINS_A_CORE = WINS_A * WINA_BINS           # 264
E_CORE = WINS_A * WINA_EDGES               # 2640
SLOTS_A_CORE = WINS_A * WINA_SLOTS         # 135168
E_PAD = NC * E_CORE                        # 21120
BINS_A = NC * BINS_A_CORE                  # 2112
CHUNK_WINS = [6, 6, 5, 5]                  # phase-A packed-table chunks
TAB_ROWS = 32768

# phase C (node side)
NB_NODES = 12        # max nodes per bin
NB_SLOTS = 128       # slots per bin (1 block)
WINC_BINS = 10       # bins per window -> 120 node columns
WINC_NODES = WINC_BINS * NB_NODES          # 120
WINC_SLOTS = WINC_BINS * NB_SLOTS          # 1280
WINS_C = 108                                # windows per core
BINS_C_CORE = WINS_C * WINC_BINS           # 1080
N_CORE = WINS_C * WINC_NODES               # 12960
SLOTS_C_CORE = WINS_C * WINC_SLOTS         # 138240
N_PAD = NC * N_CORE                        # 103680
BINS_C = NC * BINS_C_CORE                  # 8640
CALLS_C = WINS_C // 2                      # 2 windows per gather call
CALL_C_IDX = 2 * WINC_SLOTS                # 2560

PREG = 27            # phase C-pre column groups of 480 (27*480 == 12960)

F32 = np.float32


# ---------------------------------------------------------------- host prep
def _greedy_bins(counts, nbins, cap_items, cap_slots):
    """LPT-style packing of items into nbins bins with item-count and
    slot-sum caps. Returns (list of per-bin item lists, per-bin slot sums)."""
    order = np.argsort(-counts, kind="stable")
    heap = [(0, 0, b) for b in range(nbins)]
    heapq.heapify(heap)
    items = [[] for _ in range(nbins)]
    for it in order:
        c = int(counts[it])
        while True:
            if not heap:
                raise RuntimeError("bin packing failed: no bins left")
            slots, n, b = heapq.heappop(heap)
            if n < cap_items:
                break
        if slots + c > cap_slots:
            raise RuntimeError(f"bin packing failed: {slots}+{c}>{cap_slots}")
        items[b].append(int(it))
        heapq.heappush(heap, (slots + c, n + 1, b))
    sums = np.array([sum(int(counts[e]) for e in bl) for bl in items])
    return items, sums


def _snake_assign(bin_sums, nbins_per_core):
    """Assign bins to cores, balancing total slots: snake over bins sorted
    by load. Returns per-core lists of bin ids (each exactly nbins_per_core)."""
    order = np.argsort(-bin_sums, kind="stable")
    cores = [[] for _ in range(NC)]
    i = 0
    for b in order:
        # snake: 0..7, 7..0, ...
        rnd, pos = divmod(i, NC)
        c = pos if rnd % 2 == 0 else NC - 1 - pos
        # skip full cores (only near the end)
        if len(cores[c]) >= nbins_per_core:
            c = min(range(NC), key=lambda x: len(cores[x]))
        cores[c].append(int(b))
        i += 1
    return cores


def _wrap_idx(vals, ncols):
    """dma_gather index layout: flat position i -> [i%16 (replicated x8), i//16]."""
    w = vals.reshape(ncols, 16).T.astype(np.int16)
    return np.tile(w, (8, 1))


def _prep(X, X0, vertex, edges):
    rng_chk = None  # silence lints
    counts_e = np.bincount(edges, minlength=N_EDGES)
    counts_v = np.bincount(vertex, minlength=N_NODES)
    assert counts_e.max() <= EB_SLOTS, counts_e.max()
    assert counts_v.max() <= NB_SLOTS, counts_v.max()

    # incidence lists grouped by edge / by vertex
    order_e = np.argsort(edges, kind="stable")
    estart = np.zeros(N_EDGES + 1, np.int64)
    estart[1:] = np.cumsum(counts_e)
    order_v = np.argsort(vertex, kind="stable")
    vstart = np.zeros(N_NODES + 1, np.int64)
    vstart[1:] = np.cumsum(counts_v)

    ebins, esums = _greedy_bins(counts_e, BINS_A, EB_EDGES, EB_SLOTS)
    ecores = _snake_assign(esums, BINS_A_CORE)
    nbins, nsums = _greedy_bins(counts_v, BINS_C, NB_NODES, NB_SLOTS)
    ncores = _snake_assign(nsums, BINS_C_CORE)

    # new edge ids
    edge_core = np.full(N_EDGES, -1, np.int64)
    edge_local = np.full(N_EDGES, -1, np.int64)
    for c in range(NC):
        for k, b in enumerate(ecores[c]):
            w, bi = divmod(k, WINA_BINS)
            for p, e in enumerate(ebins[b]):
                edge_core[e] = c
                edge_local[e] = w * WINA_EDGES + bi * EB_EDGES + p
    assert (edge_core >= 0).all()
    new_edge_global = edge_core * E_CORE + edge_local

    node_core = np.full(N_NODES, -1, np.int64)
    node_local = np.full(N_NODES, -1, np.int64)
    for c in range(NC):
        for k, b in enumerate(ncores[c]):
            w, bi = divmod(k, WINC_BINS)
            for p, v in enumerate(nbins[b]):
                node_core[v] = c
                node_local[v] = w * WINC_NODES + bi * NB_NODES + p
    assert (node_core >= 0).all()

    chunk_of_win = []
    for k, nw in enumerate(CHUNK_WINS):
        chunk_of_win += [k] * nw
    chunk_bounds = np.cumsum([0] + [nw * WINA_SLOTS for nw in CHUNK_WINS])

    in_maps = []
    for c in range(NC):
        # ---- phase A slot stream
        slotsA = np.zeros(SLOTS_A_CORE, np.int64)
        segA = np.full(SLOTS_A_CORE, -1.0, F32)
        cnt_core = np.zeros(E_CORE, F32)
        for k, b in enumerate(ecores[c]):
            base = k * EB_SLOTS
            off = 0
            w, bi = divmod(k, WINA_BINS)
            for p, e in enumerate(ebins[b]):
                inc = order_e[estart[e]:estart[e + 1]]
                slotsA[base + off: base + off + len(inc)] = vertex[inc]
                segA[base + off: base + off + len(inc)] = p
                off += len(inc)
                cnt_core[w * WINA_EDGES + bi * EB_EDGES + p] = len(inc)

        # ---- packed tables + int16 indices per chunk
        tabs = []
        idx_vals = np.zeros(SLOTS_A_CORE, np.int64)
        for k in range(len(CHUNK_WINS)):
            a, bnd = chunk_bounds[k], chunk_bounds[k + 1]
            uniq, inv = np.unique(slotsA[a:bnd], return_inverse=True)
            assert len(uniq) <= TAB_ROWS, f"chunk {k} core {c}: {len(uniq)}"
            tab = np.zeros((TAB_ROWS, D), F32)
            tab[: len(uniq)] = X[uniq]
            tabs.append(tab)
            idx_vals[a:bnd] = inv
        idxa = np.zeros((128, SLOTS_A_CORE // 16), np.int16)
        for w in range(WINS_A):
            idxa[:, w * 384:(w + 1) * 384] = _wrap_idx(
                idx_vals[w * WINA_SLOTS:(w + 1) * WINA_SLOTS], 384)

        # ---- phase C slot stream
        slotsC = np.zeros(SLOTS_C_CORE, np.int64)
        segC = np.full(SLOTS_C_CORE, -1.0, F32)
        perm_nodes = np.zeros(N_CORE, np.int64)  # new local -> orig (dummy -> -1)
        perm_valid = np.zeros(N_CORE, bool)
        deg_core = np.zeros(N_CORE, F32)
        for k, b in enumerate(ncores[c]):
            base = k * NB_SLOTS
            off = 0
            w, bi = divmod(k, WINC_BINS)
            for p, v in enumerate(nbins[b]):
                inc = order_v[vstart[v]:vstart[v + 1]]
                slotsC[base + off: base + off + len(inc)] = \
                    new_edge_global[edges[inc]]
                segC[base + off: base + off + len(inc)] = p
                off += len(inc)
                loc = w * WINC_NODES + bi * NB_NODES + p
                perm_nodes[loc] = v
                perm_valid[loc] = True
                deg_core[loc] = len(inc)
        idxc = np.zeros((128, SLOTS_C_CORE // 16), np.int16)
        for cc in range(CALLS_C):
            idxc[:, cc * 160:(cc + 1) * 160] = _wrap_idx(
                slotsC[cc * CALL_C_IDX:(cc + 1) * CALL_C_IDX], 160)

        xp = np.zeros((N_CORE, D), F32)
        x0p = np.zeros((N_CORE, D), F32)
        xp[perm_valid] = X[perm_nodes[perm_valid]]
        x0p[perm_valid] = X0[perm_nodes[perm_valid]]

        in_map = {
            "tab0": tabs[0], "tab1": tabs[1], "tab2": tabs[2], "tab3": tabs[3],
            "idxa": idxa,
            "sega": np.ascontiguousarray(segA.reshape(-1, 128).T),
            "idxc": idxc,
            "segc": np.ascontiguousarray(segC.reshape(-1, 128).T),
            "cnt": cnt_core.reshape(1, E_CORE),
            "deg": deg_core.reshape(1, N_CORE),
            "xt": np.ascontiguousarray(xp.T),
            "x0t": np.ascontiguousarray(x0p.T),
        }
        in_maps.append(in_map)

    meta = dict(edge_core=edge_core, edge_local=edge_local,
                node_core=node_core, node_local=node_local)
    return in_maps, meta


# ---------------------------------------------------------------- device
def _build_kernel():
    import concourse.bacc as bacc
    import concourse.mybir as mybir
    import concourse.tile as tile

    f32 = mybir.dt.float32
    i16 = mybir.dt.int16
    mult = mybir.AluOpType.mult
    iseq = mybir.AluOpType.is_equal

    nc = bacc.Bacc("TRN2", target_bir_lowering=False, num_devices=NC,
                   debug=False, num_swdge_queues=4)

    # ---- dram I/O
    t_tabs = [nc.dram_tensor(f"tab{k}", [TAB_ROWS, D], f32, kind="ExternalInput")
              for k in range(4)]
    t_idxa = nc.dram_tensor("idxa", [128, SLOTS_A_CORE // 16], i16, kind="ExternalInput")
    t_sega = nc.dram_tensor("sega", [128, SLOTS_A_CORE // 128], f32, kind="ExternalInput")
    t_idxc = nc.dram_tensor("idxc", [128, SLOTS_C_CORE // 16], i16, kind="ExternalInput")
    t_segc = nc.dram_tensor("segc", [128, SLOTS_C_CORE // 128], f32, kind="ExternalInput")
    t_cnt = nc.dram_tensor("cnt", [1, E_CORE], f32, kind="ExternalInput")
    t_deg = nc.dram_tensor("deg", [1, N_CORE], f32, kind="ExternalInput")
    t_xt = nc.dram_tensor("xt", [D, N_CORE], f32, kind="ExternalInput")
    t_x0t = nc.dram_tensor("x0t", [D, N_CORE], f32, kind="ExternalInput")
    t_w1t = nc.dram_tensor("w1t", [D, D], f32, kind="ExternalInput")
    t_w2a = nc.dram_tensor("w2a", [D, D], f32, kind="ExternalInput")
    t_w2b = nc.dram_tensor("w2b", [D, D], f32, kind="ExternalInput")
    t_wwt = nc.dram_tensor("wwt", [D, D], f32, kind="ExternalInput")
    t_b1 = nc.dram_tensor("b1", [1, D], f32, kind="ExternalInput")
    t_b2c = nc.dram_tensor("b2c", [D, 1], f32, kind="ExternalInput")
    t_wb = nc.dram_tensor("wb", [1, D], f32, kind="ExternalInput")
    t_ident = nc.dram_tensor("ident", [D, D], f32, kind="ExternalInput")
    t_iotaa = nc.dram_tensor("iotaa", [128, WINA_BLOCKS * EB_EDGES], f32, kind="ExternalInput")
    t_iotac = nc.dram_tensor("iotac", [128, 20 * NB_NODES], f32, kind="ExternalInput")

    t_xe = nc.dram_tensor("xe", [E_CORE, D], f32, kind="ExternalOutput")
    t_outt = nc.dram_tensor("outt", [D, N_CORE], f32, kind="ExternalOutput")

    t_zchunk = nc.dram_tensor("z_chunk", [E_CORE, D], f32, kind="Internal")
    t_zfull = nc.dram_tensor("z_full", [E_PAD, D], f32, kind="Internal")
    t_acc = nc.dram_tensor("acc_d", [D, N_CORE], f32, kind="Internal")

    chunk_of_win = []
    for k, nw in enumerate(CHUNK_WINS):
        chunk_of_win += [k] * nw

    with tile.TileContext(nc) as tc:
        with tc.tile_pool(name="const", bufs=1) as cp:
            # resident tables
            def load(t, shape, dtype=f32):
                s = cp.tile(shape, dtype, tag=t.name)
                nc.sync.dma_start(out=s[:], in_=t[:])
                return s

            idxa_sb = load(t_idxa, [128, SLOTS_A_CORE // 16], i16)
            sega_sb = load(t_sega, [128, SLOTS_A_CORE // 128])
            idxc_sb = load(t_idxc, [128, SLOTS_C_CORE // 16], i16)
            segc_sb = load(t_segc, [128, SLOTS_C_CORE // 128])
            cnt_sb = load(t_cnt, [1, E_CORE])
            w1t_sb = load(t_w1t, [D, D])
            w2a_sb = load(t_w2a, [D, D])
            w2b_sb = load(t_w2b, [D, D])
            wwt_sb = load(t_wwt, [D, D])
            b1_sb = load(t_b1, [1, D])
            b2c_sb = load(t_b2c, [D, 1])
            wb_sb = load(t_wb, [1, D])
            ident_sb = load(t_ident, [D, D])
            iotaa_sb = load(t_iotaa, [128, WINA_BLOCKS * EB_EDGES])
            iotac_sb = load(t_iotac, [128, 20 * NB_NODES])

            ones_sb = cp.tile([1, 480], f32, tag="ones")
            nc.vector.memset(ones_sb[:], 1.0)

            g1t_sb = cp.tile([D, D], f32, tag="g1t")
            g2t_sb = cp.tile([D, D], f32, tag="g2t")
            c2_sb = cp.tile([1, D], f32, tag="c2")
            wht_sb = cp.tile([D, D], f32, tag="wht")

            with tc.tile_pool(name="psprep", bufs=1, space="PSUM") as pp:
                g1t_ps = pp.tile([D, D], f32, tag="a")
                nc.tensor.matmul(out=g1t_ps[:], lhsT=w2b_sb[:], rhs=wwt_sb[:],
                                 start=True, stop=True)
                nc.scalar.mul(out=g1t_sb[:], in_=g1t_ps[:], mul=1.0 - ALPHA)
                g2t_ps = pp.tile([D, D], f32, tag="b")
                nc.tensor.matmul(out=g2t_ps[:], lhsT=w2a_sb[:], rhs=wwt_sb[:],
                                 start=True, stop=True)
                nc.scalar.mul(out=g2t_sb[:], in_=g2t_ps[:], mul=1.0 - ALPHA)
                c2_ps = pp.tile([1, D], f32, tag="c")
                nc.tensor.matmul(out=c2_ps[:], lhsT=b2c_sb[:], rhs=wwt_sb[:],
                                 start=True, stop=True)
                nc.scalar.mul(out=c2_sb[:], in_=c2_ps[:], mul=1.0 - ALPHA)
            nc.scalar.mul(out=wht_sb[:], in_=wwt_sb[:], mul=ALPHA)

            _skip = os.environ.get("KB_SKIP", "")
            # ================= phase A =================
            with tc.tile_pool(name="ga", bufs=6) as gap, \
                 tc.tile_pool(name="sa", bufs=4) as sap, \
                 tc.tile_pool(name="sba", bufs=2) as sba, \
                 tc.tile_pool(name="psye", bufs=4, space="PSUM") as psye, \
                 tc.tile_pool(name="psepi", bufs=1, space="PSUM") as psepi:
                for w in range(0 if "A" in _skip else WINS_A):
                    gath = gap.tile([128, WINA_BLOCKS * D], f32, tag="g")
                    nc.gpsimd.dma_gather(
                        out_ap=gath[:].rearrange("p (j d) -> p j d", d=D),
                        in_ap=t_tabs[chunk_of_win[w]][:],
                        idxs_ap=idxa_sb[:, w * 384:(w + 1) * 384],
                        num_idxs=WINA_SLOTS, num_idxs_reg=WINA_SLOTS,
                        elem_size=D, single_packet=False, queue_num=w % 4)
                    s_w = sap.tile([128, WINA_BLOCKS * EB_EDGES], f32, tag="s")
                    nc.vector.tensor_tensor(
                        out=s_w[:],
                        in0=sega_sb[:, w * WINA_BLOCKS:(w + 1) * WINA_BLOCKS, None]
                            .to_broadcast([128, WINA_BLOCKS, EB_EDGES]),
                        in1=iotaa_sb[:], op=iseq)
                    yet_ps = psye.tile([D, WINA_EDGES], f32, tag="ye")
                    for g in range(WINA_BINS):
                        for j in range(EB_BLOCKS):
                            b = g * EB_BLOCKS + j
                            nc.tensor.matmul(
                                out=yet_ps[:, g * EB_EDGES:(g + 1) * EB_EDGES],
                                lhsT=gath[:, b * D:(b + 1) * D],
                                rhs=s_w[:, b * EB_EDGES:(b + 1) * EB_EDGES],
                                start=(j == 0), stop=(j == EB_BLOCKS - 1))
                    yet_sb = sba.tile([D, WINA_EDGES], f32, tag="yet")
                    nc.vector.tensor_copy(out=yet_sb[:], in_=yet_ps[:])
                    xet_ps = psepi.tile([D, WINA_EDGES], f32, tag="xet")
                    nc.tensor.matmul(out=xet_ps[:], lhsT=w1t_sb[:], rhs=yet_sb[:],
                                     start=True, stop=False)
                    nc.tensor.matmul(
                        out=xet_ps[:], lhsT=b1_sb[:],
                        rhs=cnt_sb[0:1, w * WINA_EDGES:(w + 1) * WINA_EDGES],
                        start=False, stop=True)
                    xet_sb = sba.tile([D, WINA_EDGES], f32, tag="xetc")
                    nc.scalar.copy(out=xet_sb[:], in_=xet_ps[:])
                    zt_ps = psepi.tile([D, WINA_EDGES], f32, tag="zt")
                    nc.tensor.matmul(out=zt_ps[:], lhsT=g1t_sb[:], rhs=xet_sb[:],
                                     start=True, stop=True)
                    zt_sb = sba.tile([D, WINA_EDGES], f32, tag="ztc")
                    nc.scalar.copy(out=zt_sb[:], in_=zt_ps[:])
                    xer_ps = psepi.tile([WINA_EDGES, D], f32, tag="xer")
                    nc.tensor.transpose(out=xer_ps[:], in_=xet_sb[:], identity=ident_sb[:])
                    xer_sb = sba.tile([WINA_EDGES, D], f32, tag="xerc")
                    nc.vector.tensor_copy(out=xer_sb[:], in_=xer_ps[:])
                    nc.sync.dma_start(out=t_xe[w * WINA_EDGES:(w + 1) * WINA_EDGES, :],
                                      in_=xer_sb[:])
                    zr_ps = psepi.tile([WINA_EDGES, D], f32, tag="zr")
                    nc.tensor.transpose(out=zr_ps[:], in_=zt_sb[:], identity=ident_sb[:])
                    zr_sb = sba.tile([WINA_EDGES, D], f32, tag="zrc")
                    nc.vector.tensor_copy(out=zr_sb[:], in_=zr_ps[:])
                    nc.sync.dma_start(out=t_zchunk[w * WINA_EDGES:(w + 1) * WINA_EDGES, :],
                                      in_=zr_sb[:])

            # ================= collective =================
            if "G" not in _skip:
                nc.gpsimd.collective_compute(
                "AllGather", mybir.AluOpType.bypass,
                    replica_groups=[list(range(NC))],
                    ins=[t_zchunk.ap().opt()], outs=[t_zfull.ap().opt()])

            # ================= phase C =================
            with tc.tile_pool(name="stg", bufs=3) as stg, \
                 tc.tile_pool(name="gc", bufs=6) as gcp, \
                 tc.tile_pool(name="sc", bufs=4) as scp, \
                 tc.tile_pool(name="stg2", bufs=4) as stg2, \
                 tc.tile_pool(name="pspre", bufs=1, space="PSUM") as pspre, \
                 tc.tile_pool(name="psp1", bufs=4, space="PSUM") as psp1:
                # ---- X-side terms (overlap with collective)
                for g in range(0 if "P" in _skip else PREG):
                    deg_g = stg.tile([1, 480], f32, tag="degg")
                    nc.sync.dma_start(out=deg_g[:], in_=t_deg[:, g * 480:(g + 1) * 480])
                    xt_g = stg.tile([D, 480], f32, tag="xtg")
                    nc.sync.dma_start(out=xt_g[:], in_=t_xt[:, g * 480:(g + 1) * 480])
                    x0t_g = stg.tile([D, 480], f32, tag="x0tg")
                    nc.sync.dma_start(out=x0t_g[:], in_=t_x0t[:, g * 480:(g + 1) * 480])
                    p2 = pspre.tile([D, 480], f32, tag="p2")
                    nc.tensor.matmul(out=p2[:], lhsT=g2t_sb[:], rhs=xt_g[:],
                                     start=True, stop=True)
                    p3 = pspre.tile([D, 480], f32, tag="p3")
                    nc.tensor.matmul(out=p3[:], lhsT=ones_sb[:1, :D],
                                     rhs=deg_g[:], start=True, stop=True)
                    p1x = pspre.tile([D, 480], f32, tag="p1x")
                    nc.tensor.matmul(out=p1x[:], lhsT=wht_sb[:], rhs=x0t_g[:],
                                     start=True, stop=False)
                    nc.tensor.matmul(out=p1x[:], lhsT=c2_sb[:], rhs=deg_g[:],
                                     start=False, stop=False)
                    nc.tensor.matmul(out=p1x[:], lhsT=wb_sb[:], rhs=ones_sb[:1, :480],
                                     start=False, stop=True)
                    p3sb = stg.tile([D, 480], f32, tag="p3sb")
                    nc.scalar.copy(out=p3sb[:], in_=p3[:])
                    tpre = stg.tile([D, 480], f32, tag="tpre")
                    nc.vector.tensor_tensor(out=tpre[:], in0=p2[:], in1=p3sb[:], op=mult)
                    apre = stg.tile([D, 480], f32, tag="apre")
                    nc.vector.tensor_add(out=apre[:], in0=tpre[:], in1=p1x[:])
                    nc.sync.dma_start(out=t_acc[:, g * 480:(g + 1) * 480], in_=apre[:])

                # ---- gather-Z reduce + final combine
                for c in range(0 if "C" in _skip else CALLS_C):
                    gz = gcp.tile([128, 20 * D], f32, tag="gz")
                    nc.gpsimd.dma_gather(
                        out_ap=gz[:].rearrange("p (j d) -> p j d", d=D),
                        in_ap=t_zfull[:],
                        idxs_ap=idxc_sb[:, c * 160:(c + 1) * 160],
                        num_idxs=CALL_C_IDX, num_idxs_reg=CALL_C_IDX,
                        elem_size=D, single_packet=False, queue_num=c % 4)
                    s_c = scp.tile([128, 20 * NB_NODES], f32, tag="s")
                    nc.vector.tensor_tensor(
                        out=s_c[:],
                        in0=segc_sb[:, c * 20:(c + 1) * 20, None]
                            .to_broadcast([128, 20, NB_NODES]),
                        in1=iotac_sb[:], op=iseq)
                    acc_in = stg2.tile([D, 240], f32, tag="accin")
                    nc.sync.dma_start(out=acc_in[:], in_=t_acc[:, c * 240:(c + 1) * 240])
                    out_sb = stg2.tile([D, 240], f32, tag="osb")
                    for lw in range(2):
                        p1 = psp1.tile([D, WINC_NODES], f32, tag="p1")
                        for g in range(WINC_BINS):
                            blk = lw * WINC_BINS + g
                            nc.tensor.matmul(
                                out=p1[:, g * NB_NODES:(g + 1) * NB_NODES],
                                lhsT=gz[:, blk * D:(blk + 1) * D],
                                rhs=s_c[:, blk * NB_NODES:(blk + 1) * NB_NODES],
                                start=True, stop=True)
                        nc.vector.tensor_add(
                            out=out_sb[:, lw * WINC_NODES:(lw + 1) * WINC_NODES],
                            in0=p1[:],
                            in1=acc_in[:, lw * WINC_NODES:(lw + 1) * WINC_NODES])
                    nc.sync.dma_start(out=t_outt[:, c * 240:(c + 1) * 240], in_=out_sb[:])

    nc.compile()
    return nc


# ---------------------------------------------------------------- main entry
def kernel(X, X0, vertex, edges, W1_w, W1_b, W2_w, W2_b, W_w, W_b):
    import time
    t0 = time.time()
    verbose = os.environ.get("KERNEL_VERBOSE", "0") == "1"
    trace = os.environ.get("KERNEL_TRACE", "0") == "1"

    X = np.asarray(X, F32)
    X0 = np.asarray(X0, F32)
    vertex = np.asarray(vertex).astype(np.int64)
    edges = np.asarray(edges).astype(np.int64)
    W1_w = np.asarray(W1_w, F32)
    W1_b = np.asarray(W1_b, F32)
    W2_w = np.asarray(W2_w, F32)
    W2_b = np.asarray(W2_b, F32)
    W_w = np.asarray(W_w, F32)
    W_b = np.asarray(W_b, F32)

    in_maps, meta = _prep(X, X0, vertex, edges)
    if verbose:
        print(f"[kernel] host prep: {time.time() - t0:.1f}s")

    shared = {
        "w1t": np.ascontiguousarray(W1_w.T),
        "w2a": np.ascontiguousarray(W2_w[:, :D]),
        "w2b": np.ascontiguousarray(W2_w[:, D:]),
        "wwt": np.ascontiguousarray(W_w.T),
        "b1": W1_b.reshape(1, D),
        "b2c": W2_b.reshape(D, 1),
        "wb": W_b.reshape(1, D),
        "ident": np.eye(D, dtype=F32),
        "iotaa": np.tile(np.arange(EB_EDGES, dtype=F32), (128, WINA_BLOCKS)),
        "iotac": np.tile(np.arange(NB_NODES, dtype=F32), (128, 20)),
    }
    for m in in_maps:
        m.update(shared)

    t1 = time.time()
    nc = _build_kernel()
    if verbose:
        print(f"[kernel] build+compile: {time.time() - t1:.1f}s")

    from concourse.bass_utils import run_bass_kernel_spmd
    t2 = time.time()
    res = run_bass_kernel_spmd(nc, in_maps, core_ids=list(range(NC)),
                               trace=trace,
                               trace_cores=list(range(NC)) if trace else None)
    if verbose:
        print(f"[kernel] device run: {time.time() - t2:.1f}s")
    if trace and res.exec_time_ns is not None:
        print(f"HW exec time: {res.exec_time_ns} ns")
        if res.instructions_and_trace is not None:
            print(f"trace: {res.instructions_and_trace[1]}")

    # ---- assemble
    out = np.zeros((N_NODES, D), F32)
    xe = np.zeros((N_EDGES, D), F32)
    e_ids = np.arange(N_EDGES)
    n_ids = np.arange(N_NODES)
    for c in range(NC):
        r = res.results[c]
        em = meta["edge_core"] == c
        xe[e_ids[em]] = r["xe"][meta["edge_local"][em]]
        nm = meta["node_core"] == c
        out[n_ids[nm]] = r["outt"].T[meta["node_local"][nm]]
    return out, xe


# revision 6
# speedup vs baseline: 1.0897x; 1.0897x over previous
"""EquivSetConv (hypergraph GNN message passing) on 8 Trainium2 NeuronCores.

Math (reference):
    H   = X @ W1_w.T + b1
    Xe  = segment_sum(H[vertex], edges, E)            # [E, 64]
    Xev = concat([X[vertex], Xe[edges]], -1) @ W2_w.T + b2
    Xv  = segment_sum(Xev, vertex, N)                 # [N, 64]
    out = ((1-a)*Xv + a*X0) @ W_w.T + W_b             # a = 0.5
    returns (out, Xe)

Reformulation used here (all matmuls pushed through the segment sums):
    Ye  = segment_sum(X[vertex], edges)               # gather-X + on-chip reduce
    Xe  = Ye @ W1_w.T + cnt (x) b1
    Z   = Xe @ G1.T          with G1 = (1-a) * W_w @ W2b      (W2b = W2_w[:, 64:])
    A'  = segment_sum(Z[edges-of-incidence], vertex)  # gather-Z + on-chip reduce
    out = A' + deg . (X @ G2.T) + c2 (x) deg + a * X0 @ W_w.T + 1 (x) W_b
          with G2 = (1-a) * W_w @ W2a  (W2a = W2_w[:, :64]),  c2 = (1-a) * W_w @ b2

Sharding: edges bin-packed into per-core windows (phase A), nodes bin-packed
into per-core windows (phase C). The two gathers use 4-queue dma_gather;
the segment sums are one-hot (S) matmuls on the PE into transposed PSUM
windows, with S built on-device from per-slot segment ids via broadcast
is_equal. Z is AllGathered between the phases (5.4 MB).
"""

import heapq
import os

import ml_dtypes
import numpy as np

BF16 = ml_dtypes.bfloat16

# ---------------------------------------------------------------- constants
N_NODES = 100000
N_EDGES = 20000
D = 64
NC = 8
ALPHA = 0.5

# phase A (edge side)
EB_EDGES = 10        # max edges per bin
EB_SLOTS = 512       # slots per bin (4 blocks of 128)
EB_BLOCKS = 4
WINA_BINS = 12       # bins per window -> 120 edge columns
WINA_EDGES = WINA_BINS * EB_EDGES          # 120
WINA_SLOTS = WINA_BINS * EB_SLOTS          # 6144
WINA_BLOCKS = WINA_BINS * EB_BLOCKS        # 48
WINS_A = 22                                 # windows per core
BINS_A_CORE = WINS_A * WINA_BINS           # 264
E_CORE = WINS_A * WINA_EDGES               # 2640
SLOTS_A_CORE = WINS_A * WINA_SLOTS         # 135168
E_PAD = NC * E_CORE                        # 21120
BINS_A = NC * BINS_A_CORE                  # 2112
CHUNK_WINS = [6, 6, 5, 5]                  # phase-A packed-table chunks
TAB_ROWS = 32768

# phase C (node side)
NB_NODES = 12        # max nodes per bin
NB_SLOTS = 128       # slots per bin (1 block)
WINC_BINS = 10       # bins per window -> 120 node columns
WINC_NODES = WINC_BINS * NB_NODES          # 120
WINC_SLOTS = WINC_BINS * NB_SLOTS          # 1280
WINS_C = 108                                # windows per core
BINS_C_CORE = WINS_C * WINC_BINS           # 1080
N_CORE = WINS_C * WINC_NODES               # 12960
SLOTS_C_CORE = WINS_C * WINC_SLOTS         # 138240
N_PAD = NC * N_CORE                        # 103680
BINS_C = NC * BINS_C_CORE                  # 8640
CALLS_C = WINS_C // 2                      # 2 windows per gather call
CALL_C_IDX = 2 * WINC_SLOTS                # 2560

PREG = 27            # phase C-pre column groups of 480 (27*480 == 12960)

F32 = np.float32


# ---------------------------------------------------------------- host prep
def _greedy_bins(counts, nbins, cap_items, cap_slots):
    """LPT-style packing of items into nbins bins with item-count and
    slot-sum caps. Returns (list of per-bin item lists, per-bin slot sums)."""
    order = np.argsort(-counts, kind="stable")
    heap = [(0, 0, b) for b in range(nbins)]
    heapq.heapify(heap)
    items = [[] for _ in range(nbins)]
    for it in order:
        c = int(counts[it])
        while True:
            if not heap:
                raise RuntimeError("bin packing failed: no bins left")
            slots, n, b = heapq.heappop(heap)
            if n < cap_items:
                break
        if slots + c > cap_slots:
            raise RuntimeError(f"bin packing failed: {slots}+{c}>{cap_slots}")
        items[b].append(int(it))
        heapq.heappush(heap, (slots + c, n + 1, b))
    sums = np.array([sum(int(counts[e]) for e in bl) for bl in items])
    return items, sums


def _snake_assign(bin_sums, nbins_per_core):
    """Assign bins to cores, balancing total slots: snake over bins sorted
    by load. Returns per-core lists of bin ids (each exactly nbins_per_core)."""
    order = np.argsort(-bin_sums, kind="stable")
    cores = [[] for _ in range(NC)]
    i = 0
    for b in order:
        # snake: 0..7, 7..0, ...
        rnd, pos = divmod(i, NC)
        c = pos if rnd % 2 == 0 else NC - 1 - pos
        # skip full cores (only near the end)
        if len(cores[c]) >= nbins_per_core:
            c = min(range(NC), key=lambda x: len(cores[x]))
        cores[c].append(int(b))
        i += 1
    return cores


def _wrap_idx(vals, ncols):
    """dma_gather index layout: flat position i -> [i%16 (replicated x8), i//16]."""
    w = vals.reshape(ncols, 16).T.astype(np.int16)
    return np.tile(w, (8, 1))


def _prep(X, X0, vertex, edges):
    rng_chk = None  # silence lints
    counts_e = np.bincount(edges, minlength=N_EDGES)
    counts_v = np.bincount(vertex, minlength=N_NODES)
    assert counts_e.max() <= EB_SLOTS, counts_e.max()
    assert counts_v.max() <= NB_SLOTS, counts_v.max()

    # incidence lists grouped by edge / by vertex
    order_e = np.argsort(edges, kind="stable")
    estart = np.zeros(N_EDGES + 1, np.int64)
    estart[1:] = np.cumsum(counts_e)
    order_v = np.argsort(vertex, kind="stable")
    vstart = np.zeros(N_NODES + 1, np.int64)
    vstart[1:] = np.cumsum(counts_v)

    ebins, esums = _greedy_bins(counts_e, BINS_A, EB_EDGES, EB_SLOTS)
    ecores = _snake_assign(esums, BINS_A_CORE)
    nbins, nsums = _greedy_bins(counts_v, BINS_C, NB_NODES, NB_SLOTS)
    ncores = _snake_assign(nsums, BINS_C_CORE)

    # new edge ids
    edge_core = np.full(N_EDGES, -1, np.int64)
    edge_local = np.full(N_EDGES, -1, np.int64)
    for c in range(NC):
        for k, b in enumerate(ecores[c]):
            w, bi = divmod(k, WINA_BINS)
            for p, e in enumerate(ebins[b]):
                edge_core[e] = c
                edge_local[e] = w * WINA_EDGES + bi * EB_EDGES + p
    assert (edge_core >= 0).all()
    new_edge_global = edge_core * E_CORE + edge_local

    node_core = np.full(N_NODES, -1, np.int64)
    node_local = np.full(N_NODES, -1, np.int64)
    for c in range(NC):
        for k, b in enumerate(ncores[c]):
            w, bi = divmod(k, WINC_BINS)
            for p, v in enumerate(nbins[b]):
                node_core[v] = c
                node_local[v] = w * WINC_NODES + bi * NB_NODES + p
    assert (node_core >= 0).all()

    chunk_of_win = []
    for k, nw in enumerate(CHUNK_WINS):
        chunk_of_win += [k] * nw
    chunk_bounds = np.cumsum([0] + [nw * WINA_SLOTS for nw in CHUNK_WINS])

    in_maps = []
    for c in range(NC):
        # ---- phase A slot stream
        slotsA = np.zeros(SLOTS_A_CORE, np.int64)
        segA = np.full(SLOTS_A_CORE, -1.0, F32)
        cnt_core = np.zeros(E_CORE, F32)
        for k, b in enumerate(ecores[c]):
            base = k * EB_SLOTS
            off = 0
            w, bi = divmod(k, WINA_BINS)
            for p, e in enumerate(ebins[b]):
                inc = order_e[estart[e]:estart[e + 1]]
                slotsA[base + off: base + off + len(inc)] = vertex[inc]
                segA[base + off: base + off + len(inc)] = p
                off += len(inc)
                cnt_core[w * WINA_EDGES + bi * EB_EDGES + p] = len(inc)

        # ---- packed tables + int16 indices per chunk
        tabs = []
        idx_vals = np.zeros(SLOTS_A_CORE, np.int64)
        for k in range(len(CHUNK_WINS)):
            a, bnd = chunk_bounds[k], chunk_bounds[k + 1]
            uniq, inv = np.unique(slotsA[a:bnd], return_inverse=True)
            assert len(uniq) <= TAB_ROWS, f"chunk {k} core {c}: {len(uniq)}"
            tab = np.zeros((TAB_ROWS, D), F32)
            tab[: len(uniq)] = X[uniq]
            tabs.append(tab)
            idx_vals[a:bnd] = inv
        idxa = np.zeros((128, SLOTS_A_CORE // 16), np.int16)
        for w in range(WINS_A):
            idxa[:, w * 384:(w + 1) * 384] = _wrap_idx(
                idx_vals[w * WINA_SLOTS:(w + 1) * WINA_SLOTS], 384)

        # ---- phase C slot stream
        slotsC = np.zeros(SLOTS_C_CORE, np.int64)
        segC = np.full(SLOTS_C_CORE, -1.0, F32)
        perm_nodes = np.zeros(N_CORE, np.int64)  # new local -> orig (dummy -> -1)
        perm_valid = np.zeros(N_CORE, bool)
        deg_core = np.zeros(N_CORE, F32)
        for k, b in enumerate(ncores[c]):
            base = k * NB_SLOTS
            off = 0
            w, bi = divmod(k, WINC_BINS)
            for p, v in enumerate(nbins[b]):
                inc = order_v[vstart[v]:vstart[v + 1]]
                slotsC[base + off: base + off + len(inc)] = \
                    new_edge_global[edges[inc]]
                segC[base + off: base + off + len(inc)] = p
                off += len(inc)
                loc = w * WINC_NODES + bi * NB_NODES + p
                perm_nodes[loc] = v
                perm_valid[loc] = True
                deg_core[loc] = len(inc)
        idxc = np.zeros((128, SLOTS_C_CORE // 16), np.int16)
        for cc in range(CALLS_C):
            idxc[:, cc * 160:(cc + 1) * 160] = _wrap_idx(
                slotsC[cc * CALL_C_IDX:(cc + 1) * CALL_C_IDX], 160)

        xp = np.zeros((N_CORE, D), F32)
        x0p = np.zeros((N_CORE, D), F32)
        xp[perm_valid] = X[perm_nodes[perm_valid]]
        x0p[perm_valid] = X0[perm_nodes[perm_valid]]

        in_map = {
            "tab0": tabs[0], "tab1": tabs[1], "tab2": tabs[2], "tab3": tabs[3],
            "idxa": idxa,
            "sega": np.ascontiguousarray(segA.reshape(-1, 128).T).astype(BF16),
            "idxc": idxc,
            "segc": np.ascontiguousarray(segC.reshape(-1, 128).T).astype(BF16),
            "cnt": cnt_core.reshape(1, E_CORE).astype(BF16),
            "deg": deg_core.reshape(1, N_CORE).astype(BF16),
            "xt": np.ascontiguousarray(xp.T).astype(BF16),
            "x0t": np.ascontiguousarray(x0p.T).astype(BF16),
        }
        in_maps.append(in_map)

    meta = dict(edge_core=edge_core, edge_local=edge_local,
                node_core=node_core, node_local=node_local)
    return in_maps, meta


# ---------------------------------------------------------------- device
def _build_kernel():
    import concourse.bacc as bacc
    import concourse.mybir as mybir
    import concourse.tile as tile

    f32 = mybir.dt.float32
    bf16 = mybir.dt.bfloat16
    i16 = mybir.dt.int16
    mult = mybir.AluOpType.mult
    iseq = mybir.AluOpType.is_equal

    nc = bacc.Bacc("TRN2", target_bir_lowering=False, num_devices=NC,
                   debug=False, num_swdge_queues=4)

    # ---- dram I/O
    t_tabs = [nc.dram_tensor(f"tab{k}", [TAB_ROWS, D], f32, kind="ExternalInput")
              for k in range(4)]
    t_idxa = nc.dram_tensor("idxa", [128, SLOTS_A_CORE // 16], i16, kind="ExternalInput")
    t_sega = nc.dram_tensor("sega", [128, SLOTS_A_CORE // 128], bf16, kind="ExternalInput")
    t_idxc = nc.dram_tensor("idxc", [128, SLOTS_C_CORE // 16], i16, kind="ExternalInput")
    t_segc = nc.dram_tensor("segc", [128, SLOTS_C_CORE // 128], bf16, kind="ExternalInput")
    t_cnt = nc.dram_tensor("cnt", [1, E_CORE], bf16, kind="ExternalInput")
    t_deg = nc.dram_tensor("deg", [1, N_CORE], bf16, kind="ExternalInput")
    t_xt = nc.dram_tensor("xt", [D, N_CORE], bf16, kind="ExternalInput")
    t_x0t = nc.dram_tensor("x0t", [D, N_CORE], bf16, kind="ExternalInput")
    t_w1t = nc.dram_tensor("w1t", [D, D], bf16, kind="ExternalInput")
    t_w2a = nc.dram_tensor("w2a", [D, D], f32, kind="ExternalInput")
    t_w2b = nc.dram_tensor("w2b", [D, D], f32, kind="ExternalInput")
    t_wwt = nc.dram_tensor("wwt", [D, D], f32, kind="ExternalInput")
    t_b1 = nc.dram_tensor("b1", [1, D], bf16, kind="ExternalInput")
    t_b2c = nc.dram_tensor("b2c", [D, 1], f32, kind="ExternalInput")
    t_wb = nc.dram_tensor("wb", [1, D], bf16, kind="ExternalInput")
    t_ident = nc.dram_tensor("ident", [D, D], f32, kind="ExternalInput")
    t_iotaa = nc.dram_tensor("iotaa", [128, WINA_BLOCKS * EB_EDGES], bf16, kind="ExternalInput")
    t_iotac = nc.dram_tensor("iotac", [128, 20 * NB_NODES], bf16, kind="ExternalInput")

    t_xe = nc.dram_tensor("xe", [E_CORE, D], f32, kind="ExternalOutput")
    t_outt = nc.dram_tensor("outt", [D, N_CORE], f32, kind="ExternalOutput")

    t_zchunk = nc.dram_tensor("z_chunk", [E_CORE, D], f32, kind="Internal")
    t_zfull = nc.dram_tensor("z_full", [E_PAD, D], f32, kind="Internal")
    t_acc = nc.dram_tensor("acc_d", [D, N_CORE], f32, kind="Internal")

    chunk_of_win = []
    for k, nw in enumerate(CHUNK_WINS):
        chunk_of_win += [k] * nw

    with tile.TileContext(nc) as tc:
        with tc.tile_pool(name="const", bufs=1) as cp:
            # resident tables
            def load(t, shape, dtype=f32):
                s = cp.tile(shape, dtype, tag=t.name)
                nc.sync.dma_start(out=s[:], in_=t[:])
                return s

            idxa_sb = load(t_idxa, [128, SLOTS_A_CORE // 16], i16)
            sega_sb = load(t_sega, [128, SLOTS_A_CORE // 128], bf16)
            idxc_sb = load(t_idxc, [128, SLOTS_C_CORE // 16], i16)
            segc_sb = load(t_segc, [128, SLOTS_C_CORE // 128], bf16)
            cnt_sb = load(t_cnt, [1, E_CORE], bf16)
            w1t_sb = load(t_w1t, [D, D], bf16)
            w2a_sb = load(t_w2a, [D, D])
            w2b_sb = load(t_w2b, [D, D])
            wwt_sb = load(t_wwt, [D, D])
            b1_sb = load(t_b1, [1, D], bf16)
            b2c_sb = load(t_b2c, [D, 1])
            wb_sb = load(t_wb, [1, D], bf16)
            ident_sb = load(t_ident, [D, D])
            iotaa_sb = load(t_iotaa, [128, WINA_BLOCKS * EB_EDGES], bf16)
            iotac_sb = load(t_iotac, [128, 20 * NB_NODES], bf16)

            ones_sb = cp.tile([1, 480], bf16, tag="ones")
            nc.vector.memset(ones_sb[:], 1.0)

            g1t_sb = cp.tile([D, D], bf16, tag="g1t")
            g2t_sb = cp.tile([D, D], bf16, tag="g2t")
            c2_sb = cp.tile([1, D], bf16, tag="c2")
            wht_sb = cp.tile([D, D], bf16, tag="wht")

            with tc.tile_pool(name="psprep", bufs=1, space="PSUM") as pp:
                g1t_ps = pp.tile([D, D], f32, tag="a")
                nc.tensor.matmul(out=g1t_ps[:], lhsT=w2b_sb[:], rhs=wwt_sb[:],
                                 start=True, stop=True)
                nc.scalar.mul(out=g1t_sb[:], in_=g1t_ps[:], mul=1.0 - ALPHA)
                g2t_ps = pp.tile([D, D], f32, tag="b")
                nc.tensor.matmul(out=g2t_ps[:], lhsT=w2a_sb[:], rhs=wwt_sb[:],
                                 start=True, stop=True)
                nc.scalar.mul(out=g2t_sb[:], in_=g2t_ps[:], mul=1.0 - ALPHA)
                c2_ps = pp.tile([1, D], f32, tag="c")
                nc.tensor.matmul(out=c2_ps[:], lhsT=b2c_sb[:], rhs=wwt_sb[:],
                                 start=True, stop=True)
                nc.scalar.mul(out=c2_sb[:], in_=c2_ps[:], mul=1.0 - ALPHA)
            nc.scalar.mul(out=wht_sb[:], in_=wwt_sb[:], mul=ALPHA)

            _skip = os.environ.get("KB_SKIP", "")
            # ================= phase A =================
            with tc.tile_pool(name="ga", bufs=6) as gap, \
                 tc.tile_pool(name="sa", bufs=4) as sap, \
                 tc.tile_pool(name="sba", bufs=2) as sba, \
                 tc.tile_pool(name="psye", bufs=4, space="PSUM") as psye, \
                 tc.tile_pool(name="psepi", bufs=1, space="PSUM") as psepi:
                for w in range(0 if "A" in _skip else WINS_A):
                    gath = gap.tile([128, WINA_BLOCKS * D], f32, tag="g")
                    nc.gpsimd.dma_gather(
                        out_ap=gath[:].rearrange("p (j d) -> p j d", d=D),
                        in_ap=t_tabs[chunk_of_win[w]][:],
                        idxs_ap=idxa_sb[:, w * 384:(w + 1) * 384],
                        num_idxs=WINA_SLOTS, num_idxs_reg=WINA_SLOTS,
                        elem_size=D, single_packet=False, queue_num=w % 4)
                    gath_bf = gap.tile([128, WINA_BLOCKS * D], bf16, tag="gbf")
                    nc.scalar.copy(out=gath_bf[:], in_=gath[:])
                    s_w = sap.tile([128, WINA_BLOCKS * EB_EDGES], bf16, tag="s")
                    nc.vector.tensor_tensor(
                        out=s_w[:],
                        in0=sega_sb[:, w * WINA_BLOCKS:(w + 1) * WINA_BLOCKS, None]
                            .to_broadcast([128, WINA_BLOCKS, EB_EDGES]),
                        in1=iotaa_sb[:], op=iseq)
                    yet_ps = psye.tile([D, WINA_EDGES], f32, tag="ye")
                    for g in range(WINA_BINS):
                        for j in range(EB_BLOCKS):
                            b = g * EB_BLOCKS + j
                            nc.tensor.matmul(
                                out=yet_ps[:, g * EB_EDGES:(g + 1) * EB_EDGES],
                                lhsT=gath_bf[:, b * D:(b + 1) * D],
                                rhs=s_w[:, b * EB_EDGES:(b + 1) * EB_EDGES],
                                start=(j == 0), stop=(j == EB_BLOCKS - 1))
                    yet_sb = sba.tile([D, WINA_EDGES], bf16, tag="yet")
                    nc.vector.tensor_copy(out=yet_sb[:], in_=yet_ps[:])
                    xet_ps = psepi.tile([D, WINA_EDGES], f32, tag="xet")
                    nc.tensor.matmul(out=xet_ps[:], lhsT=w1t_sb[:], rhs=yet_sb[:],
                                     start=True, stop=False)
                    nc.tensor.matmul(
                        out=xet_ps[:], lhsT=b1_sb[:],
                        rhs=cnt_sb[0:1, w * WINA_EDGES:(w + 1) * WINA_EDGES],
                        start=False, stop=True)
                    xet_sb = sba.tile([D, WINA_EDGES], f32, tag="xetc")
                    nc.scalar.copy(out=xet_sb[:], in_=xet_ps[:])
                    xet_bf = sba.tile([D, WINA_EDGES], bf16, tag="xetb")
                    nc.vector.tensor_copy(out=xet_bf[:], in_=xet_ps[:])
                    zt_ps = psepi.tile([D, WINA_EDGES], f32, tag="zt")
                    nc.tensor.matmul(out=zt_ps[:], lhsT=g1t_sb[:], rhs=xet_bf[:],
                                     start=True, stop=True)
                    zt_sb = sba.tile([D, WINA_EDGES], f32, tag="ztc")
                    nc.scalar.copy(out=zt_sb[:], in_=zt_ps[:])
                    xer_ps = psepi.tile([WINA_EDGES, D], f32, tag="xer")
                    nc.tensor.transpose(out=xer_ps[:], in_=xet_sb[:], identity=ident_sb[:])
                    xer_sb = sba.tile([WINA_EDGES, D], f32, tag="xerc")
                    nc.vector.tensor_copy(out=xer_sb[:], in_=xer_ps[:])
                    nc.sync.dma_start(out=t_xe[w * WINA_EDGES:(w + 1) * WINA_EDGES, :],
                                      in_=xer_sb[:])
                    zr_ps = psepi.tile([WINA_EDGES, D], f32, tag="zr")
                    nc.tensor.transpose(out=zr_ps[:], in_=zt_sb[:], identity=ident_sb[:])
                    zr_sb = sba.tile([WINA_EDGES, D], f32, tag="zrc")
                    nc.vector.tensor_copy(out=zr_sb[:], in_=zr_ps[:])
                    nc.sync.dma_start(out=t_zchunk[w * WINA_EDGES:(w + 1) * WINA_EDGES, :],
                                      in_=zr_sb[:])

            # ================= collective =================
            if "G" not in _skip:
                nc.gpsimd.collective_compute(
                "AllGather", mybir.AluOpType.bypass,
                    replica_groups=[list(range(NC))],
                    ins=[t_zchunk.ap().opt()], outs=[t_zfull.ap().opt()])

            # ================= phase C =================
            with tc.tile_pool(name="stg", bufs=3) as stg, \
                 tc.tile_pool(name="gc", bufs=6) as gcp, \
                 tc.tile_pool(name="sc", bufs=4) as scp, \
                 tc.tile_pool(name="stg2", bufs=4) as stg2, \
                 tc.tile_pool(name="pspre", bufs=1, space="PSUM") as pspre, \
                 tc.tile_pool(name="psp1", bufs=4, space="PSUM") as psp1:
                # ---- X-side terms (overlap with collective)
                for g in range(0 if "P" in _skip else PREG):
                    deg_g = stg.tile([1, 480], bf16, tag="degg")
                    nc.sync.dma_start(out=deg_g[:], in_=t_deg[:, g * 480:(g + 1) * 480])
                    xt_g = stg.tile([D, 480], bf16, tag="xtg")
                    nc.sync.dma_start(out=xt_g[:], in_=t_xt[:, g * 480:(g + 1) * 480])
                    x0t_g = stg.tile([D, 480], bf16, tag="x0tg")
                    nc.sync.dma_start(out=x0t_g[:], in_=t_x0t[:, g * 480:(g + 1) * 480])
                    p2 = pspre.tile([D, 480], f32, tag="p2")
                    nc.tensor.matmul(out=p2[:], lhsT=g2t_sb[:], rhs=xt_g[:],
                                     start=True, stop=True)
                    p3 = pspre.tile([D, 480], f32, tag="p3")
                    nc.tensor.matmul(out=p3[:], lhsT=ones_sb[:1, :D],
                                     rhs=deg_g[:], start=True, stop=True)
                    p1x = pspre.tile([D, 480], f32, tag="p1x")
                    nc.tensor.matmul(out=p1x[:], lhsT=wht_sb[:], rhs=x0t_g[:],
                                     start=True, stop=False)
                    nc.tensor.matmul(out=p1x[:], lhsT=c2_sb[:], rhs=deg_g[:],
                                     start=False, stop=False)
                    nc.tensor.matmul(out=p1x[:], lhsT=wb_sb[:], rhs=ones_sb[:1, :480],
                                     start=False, stop=True)
                    p3sb = stg.tile([D, 480], f32, tag="p3sb")
                    nc.scalar.copy(out=p3sb[:], in_=p3[:])
                    tpre = stg.tile([D, 480], f32, tag="tpre")
                    nc.vector.tensor_tensor(out=tpre[:], in0=p2[:], in1=p3sb[:], op=mult)
                    apre = stg.tile([D, 480], f32, tag="apre")
                    nc.vector.tensor_add(out=apre[:], in0=tpre[:], in1=p1x[:])
                    nc.sync.dma_start(out=t_acc[:, g * 480:(g + 1) * 480], in_=apre[:])

                # ---- gather-Z reduce + final combine
                for c in range(0 if "C" in _skip else CALLS_C):
                    gz = gcp.tile([128, 20 * D], f32, tag="gz")
                    nc.gpsimd.dma_gather(
                        out_ap=gz[:].rearrange("p (j d) -> p j d", d=D),
                        in_ap=t_zfull[:],
                        idxs_ap=idxc_sb[:, c * 160:(c + 1) * 160],
                        num_idxs=CALL_C_IDX, num_idxs_reg=CALL_C_IDX,
                        elem_size=D, single_packet=False, queue_num=c % 4)
                    gz_bf = gcp.tile([128, 20 * D], bf16, tag="gzbf")
                    nc.scalar.copy(out=gz_bf[:], in_=gz[:])
                    s_c = scp.tile([128, 20 * NB_NODES], bf16, tag="s")
                    nc.vector.tensor_tensor(
                        out=s_c[:],
                        in0=segc_sb[:, c * 20:(c + 1) * 20, None]
                            .to_broadcast([128, 20, NB_NODES]),
                        in1=iotac_sb[:], op=iseq)
                    acc_in = stg2.tile([D, 240], f32, tag="accin")
                    nc.sync.dma_start(out=acc_in[:], in_=t_acc[:, c * 240:(c + 1) * 240])
                    out_sb = stg2.tile([D, 240], f32, tag="osb")
                    for lw in range(2):
                        p1 = psp1.tile([D, WINC_NODES], f32, tag="p1")
                        for g in range(WINC_BINS):
                            blk = lw * WINC_BINS + g
                            nc.tensor.matmul(
                                out=p1[:, g * NB_NODES:(g + 1) * NB_NODES],
                                lhsT=gz_bf[:, blk * D:(blk + 1) * D],
                                rhs=s_c[:, blk * NB_NODES:(blk + 1) * NB_NODES],
                                start=True, stop=True)
                        nc.vector.tensor_add(
                            out=out_sb[:, lw * WINC_NODES:(lw + 1) * WINC_NODES],
                            in0=p1[:],
                            in1=acc_in[:, lw * WINC_NODES:(lw + 1) * WINC_NODES])
                    nc.sync.dma_start(out=t_outt[:, c * 240:(c + 1) * 240], in_=out_sb[:])

    nc.compile()
    return nc


# ---------------------------------------------------------------- main entry
def kernel(X, X0, vertex, edges, W1_w, W1_b, W2_w, W2_b, W_w, W_b):
    import time
    t0 = time.time()
    verbose = os.environ.get("KERNEL_VERBOSE", "0") == "1"
    trace = os.environ.get("KERNEL_TRACE", "0") == "1"

    X = np.asarray(X, F32)
    X0 = np.asarray(X0, F32)
    vertex = np.asarray(vertex).astype(np.int64)
    edges = np.asarray(edges).astype(np.int64)
    W1_w = np.asarray(W1_w, F32)
    W1_b = np.asarray(W1_b, F32)
    W2_w = np.asarray(W2_w, F32)
    W2_b = np.asarray(W2_b, F32)
    W_w = np.asarray(W_w, F32)
    W_b = np.asarray(W_b, F32)

    in_maps, meta = _prep(X, X0, vertex, edges)
    if verbose:
        print(f"[kernel] host prep: {time.time() - t0:.1f}s")

    shared = {
        "w1t": np.ascontiguousarray(W1_w.T).astype(BF16),
        "w2a": np.ascontiguousarray(W2_w[:, :D]),
        "w2b": np.ascontiguousarray(W2_w[:, D:]),
        "wwt": np.ascontiguousarray(W_w.T),
        "b1": W1_b.reshape(1, D).astype(BF16),
        "b2c": W2_b.reshape(D, 1),
        "wb": W_b.reshape(1, D).astype(BF16),
        "ident": np.eye(D, dtype=F32),
        "iotaa": np.tile(np.arange(EB_EDGES, dtype=BF16), (128, WINA_BLOCKS)),
        "iotac": np.tile(np.arange(NB_NODES, dtype=BF16), (128, 20)),
    }
    for m in in_maps:
        m.update(shared)

    t1 = time.time()
    nc = _build_kernel()
    if verbose:
        print(f"[kernel] build+compile: {time.time() - t1:.1f}s")

    from concourse.bass_utils import run_bass_kernel_spmd
    t2 = time.time()
    res = run_bass_kernel_spmd(nc, in_maps, core_ids=list(range(NC)),
                               trace=trace,
                               trace_cores=list(range(NC)) if trace else None)
    if verbose:
        print(f"[kernel] device run: {time.time() - t2:.1f}s")
    if trace and res.exec_time_ns is not None:
        print(f"HW exec time: {res.exec_time_ns} ns")
        if res.instructions_and_trace is not None:
            print(f"trace: {res.instructions_and_trace[1]}")

    # ---- assemble
    out = np.zeros((N_NODES, D), F32)
    xe = np.zeros((N_EDGES, D), F32)
    e_ids = np.arange(N_EDGES)
    n_ids = np.arange(N_NODES)
    for c in range(NC):
        r = res.results[c]
        em = meta["edge_core"] == c
        xe[e_ids[em]] = r["xe"][meta["edge_local"][em]]
        nm = meta["node_core"] == c
        out[n_ids[nm]] = r["outt"].T[meta["node_local"][nm]]
    return out, xe
